# revision 1
# baseline (speedup 1.0000x reference)
"""Locally-connected 2D layer on 8 Trainium2 NeuronCores.

Problem: x[128,3,64,64] f32, per-position weights W[60,60,32,75], bias b[60,60,32]
  out[b,o,y,x] = sum_k patches[b,y,x,k] * W[y,x,o,k] + b[y,x,o],  k=(c,dy,dx)

Strategy (spatial sharding over output rows, 8 rows/core, memory-regime):
  - The contraction (c,dy,dx)=75 must live on SBUF partitions for the PE. dy is
    handled with a mod-5 ring of "patch planes" XP[(r%5, c, dx), x*128+b]; the
    per-row dy rotation is folded into the HOST-side W layout (np.roll), so the
    device always reads XP[0:76] as one contiguous partition range.
  - Ring planes are pre-replicated on the HOST (dx-im2col) into xpr[12,15,FXB]
    so every device fill is a plain [15, 30KB] DRAM->SBUF slice copy; fills are
    split into 4 free-chunks gated on the matmul chunks that last read the
    slot, so the ring advance overlaps the row's own compute.
  - Bias is folded in as contraction row 75 (W row 75 = bias, XP row 75 = 1.0).
  - Per output row: 15 groups of 4 column-tiled matmuls (lhsT=W[76,32],
    rhs=XP[76,128] -> out[32o,128b] at PSUM partitions 32j), PSUM->SBUF via DVE,
    one 983KB store per row in a DMA-friendly layout; host re-transposes once.
"""

import numpy as np

B, C, H, WIDTH = 128, 3, 64, 64
KH = KW = 5
RY = RX = 60
O = 32
K = 75
NCORES = 8
RPC = 8             # output rows computed per core (8*8=64, last 4 dropped)
INR = RPC + KH - 1  # 12 input rows per core
PADH = NCORES * RPC + KH - 1  # 68
NG = 15             # groups of 4 x-positions per row
CHUNKS = ((0, 4), (4, 4), (8, 4), (12, 3))  # (first group, n groups) per PSUM chunk
FXB = RX * B        # 7680 elements per patch plane

_cache = {}

USE_BF16 = True  # inputs (x-planes, W) in bf16; accumulation + output stay f32


def _build():
    import concourse.bass as bass
    import concourse.bacc as bacc
    import concourse.tile as tile
    import concourse.mybir as mybir

    f32 = mybir.dt.float32
    din = mybir.dt.bfloat16 if USE_BF16 else f32
    nc = bacc.Bacc("TRN2", target_bir_lowering=False, debug=False,
                   num_devices=NCORES)
    xpr_d = nc.dram_tensor("xpr", [INR, KH * C, FXB], din, kind="ExternalInput")
    wh_d = nc.dram_tensor("wh", [RPC, K + 1, RX, O], din, kind="ExternalInput")
    ones_d = nc.dram_tensor("ones", [1, FXB], din, kind="ExternalInput")
    oc_d = nc.dram_tensor("oc", [RPC, 4, O, NG, B], f32, kind="ExternalOutput")

    NPL = KH * C  # 15 planes per input row

    with tile.TileContext(nc) as tc:
        with (
            tc.tile_pool(name="const", bufs=1) as cpool,
            tc.tile_pool(name="w", bufs=4) as wpool,
            tc.tile_pool(name="os", bufs=2) as opool,
            tc.tile_pool(name="ps", bufs=4, space=bass.MemorySpace.PSUM) as ppool,
        ):
            xp = cpool.tile([K + 1, FXB], din)  # [76, 7680]; row 75 = ones

            nc.sync.dma_start(xp[K:K + 1, :], ones_d[:])
            for r in range(KH):  # initial ring: rows 0..4 -> slots 0..4
                nc.gpsimd.dma_start(xp[r * NPL:(r + 1) * NPL, :], xpr_d[r])

            wts = {}

            def load_w(k):
                wts[k] = wpool.tile([K + 1, RX * O], din, name="wt", tag="wt")
                nc.gpsimd.dma_start(wts[k][:],
                                    wh_d[k].rearrange("k x o -> k (x o)"))

            load_w(0)
            load_w(1)

            for k in range(RPC):
                wt = wts.pop(k)
                ot = opool.tile([128, NG * B], f32)  # [128, 1920]
                for ci, (g0, gn) in enumerate(CHUNKS):
                    pt = ppool.tile([128, 4 * B], f32)
                    for gs in range(gn):
                        for j in range(4):
                            xpos = (g0 + gs) * 4 + j
                            nc.tensor.matmul(
                                pt[32 * j:32 * (j + 1), gs * B:(gs + 1) * B],
                                wt[:, xpos * O:(xpos + 1) * O],
                                xp[:, xpos * B:(xpos + 1) * B],
                                tile_position=(0, 32 * j),
                            )
                    nc.vector.tensor_copy(
                        ot[:, g0 * B:(g0 + gn) * B], pt[:, :gn * B])
                    if k + KH < INR:
                        # ring advance for row k+1: overwrite slot k%5 with
                        # input row k+5, chunk-gated on this chunk's matmuls
                        slot = k % KH
                        f0, f1 = g0 * 4 * B, (g0 + gn) * 4 * B
                        nc.gpsimd.dma_start(
                            xp[slot * NPL:(slot + 1) * NPL, f0:f1],
                            xpr_d[k + KH, :, f0:f1])
                if k + 2 < RPC:
                    load_w(k + 2)
                nc.scalar.dma_start(
                    oc_d[k].rearrange("j o g b -> (j o) (g b)"), ot[:])

    nc.compile()
    return nc


def _get_nc():
    if "nc" not in _cache:
        _cache["nc"] = _build()
    return _cache["nc"]


def _prep_inputs(x, W, b):
    x = np.asarray(x, np.float32)
    W = np.asarray(W, np.float32)
    b = np.asarray(b, np.float32)
    xh = np.zeros((PADH, C, WIDTH, B), np.float32)
    xh[:H] = x.transpose(2, 1, 3, 0)  # [row, c, w, batch]
    # ring planes: xpr_full[r, (c,dx) -> c*KW+dx, x, b] = xh[r, c, x+dx, b]
    # plane order within a slot must be p2 = c*KW + dx (with slot-major rm)
    xpr_full = np.zeros((PADH, C, KW, RX, B), np.float32)
    for dx in range(KW):
        xpr_full[:, :, dx] = xh[:, :, dx:dx + RX]
    xpr_full = xpr_full.reshape(PADH, C * KW, FXB)
    Wfull = W.transpose(0, 3, 1, 2)  # [RY, K, RX, O]
    in_maps = []
    for i in range(NCORES):
        whc = np.zeros((RPC, K + 1, RX, O), np.float32)
        for k in range(RPC):
            y = RPC * i + k
            if y < RY:
                w5 = Wfull[y].reshape(C, KH, KW, RX, O)
                # device slot rm holds input row with (local row)%5 == rm;
                # slot rm supplies dy=(rm-k)%5 for output row k -> roll by k.
                # partition order: p = rm*15 + c*5 + dx
                whc[k, :K] = np.roll(w5, k, axis=1).transpose(1, 0, 2, 3, 4) \
                    .reshape(K, RX, O)
                whc[k, K] = b[y]
        if USE_BF16:
            import ml_dtypes
            bf = ml_dtypes.bfloat16
            in_maps.append({
                "xpr": np.ascontiguousarray(
                    xpr_full[RPC * i:RPC * i + INR]).astype(bf),
                "wh": whc.astype(bf),
                "ones": np.ones((1, FXB), bf),
            })
        else:
            in_maps.append({
                "xpr": np.ascontiguousarray(xpr_full[RPC * i:RPC * i + INR]),
                "wh": whc,
                "ones": np.ones((1, FXB), np.float32),
            })
    return in_maps


def kernel(x, W, b):
    from concourse.bass_utils import run_bass_kernel_spmd

    nc = _get_nc()
    in_maps = _prep_inputs(x, W, b)
    br = run_bass_kernel_spmd(nc, in_maps, list(range(NCORES)),
                              **_cache.get("run_kwargs", {}))
    _cache["last_run"] = br
    oc = np.stack([np.asarray(br.results[i]["oc"]) for i in range(NCORES)])
    oc = oc.reshape(NCORES * RPC, 4, O, NG, B)  # [64, j, o, x4, b]
    out = oc.transpose(4, 2, 0, 3, 1).reshape(B, O, NCORES * RPC, RX)
    return np.ascontiguousarray(out[:, :, :RY, :])



# revision 8
# speedup vs baseline: 1.0544x; 1.0544x over previous
"""Locally-connected 2D layer on 8 Trainium2 NeuronCores.

Problem: x[128,3,64,64] f32, per-position weights W[60,60,32,75], bias b[60,60,32]
  out[b,o,y,x] = sum_k patches[b,y,x,k] * W[y,x,o,k] + b[y,x,o],  k=(c,dy,dx)

Strategy (spatial sharding over output rows, 8 rows/core, memory-regime):
  - mod-6 ring of input-row "patch planes" on SBUF partitions 0..89 (6 slots x
    15 (c,dx)-planes), ones row at partition 90 -> contraction K=91 with the
    per-row dy rotation and 15 zero rows folded into the HOST-side W layout.
  - Rows are processed in PAIRS: a pair (2m, 2m+1) reads all 6 slots (input
    rows 2m..2m+5), so only 3 ring advances are needed (2 rows each), each a
    SBUF->SBUF copy from a fully prefetched staging buffer, column-chunk-gated
    so it hides under the next pair's ~3us of matmuls.
  - All HBM traffic is a handful of large HWDGE DMAs (sync queue for loads +
    ring advances, scalar queue for stores); gpsimd/SWDGE is never used.
  - Bias folded in as contraction row 90 (W row 90 = bias, XP row 90 = 1.0).
  - Per output row: 15 groups of 4 column-tiled matmuls (lhsT=W[91,32],
    rhs=XP[91,128] -> out[32o,128b] at PSUM partitions 32j), PSUM->SBUF copies
    alternate DVE/ACT engines and convert f32->bf16; one bf16 store per row.
"""

import numpy as np

B, C, H, WIDTH = 128, 3, 64, 64
KH = KW = 5
RY = RX = 60
O = 32
K = 75
NCORES = 8
RPC = 8             # output rows computed per core (8*8=64, last 4 dropped)
NSLOT = 6           # ring slots (mod-6); contraction = 6*15 + 1(ones) = 91
NPL = KW * C        # 15 planes per input row
KC = NSLOT * NPL + 1  # 91 contraction rows
PADH = NCORES * RPC + KH - 1  # 68
NG = 15             # groups of 4 x-positions per row
CHUNKS = ((0, 4), (4, 4), (8, 4), (12, 3))  # (first group, n groups) per PSUM chunk
FXB = RX * B        # 7680 elements per patch plane

_cache = {}


def _build():
    import concourse.bass as bass
    import concourse.bacc as bacc
    import concourse.tile as tile
    import concourse.mybir as mybir

    f32 = mybir.dt.float32
    din = mybir.dt.bfloat16
    nc = bacc.Bacc("TRN2", target_bir_lowering=False, debug=False,
                   num_devices=NCORES)
    x0_d = nc.dram_tensor("x0", [KC, FXB], din, kind="ExternalInput")
    x1_d = nc.dram_tensor("x1", [NSLOT * NPL, FXB], din, kind="ExternalInput")
    wh_d = nc.dram_tensor("wh", [KC, RPC * RX * O], din, kind="ExternalInput")
    oc_d = nc.dram_tensor("oc", [RPC, 128, NG * B], din, kind="ExternalOutput")

    with tile.TileContext(nc) as tc:
        with (
            tc.tile_pool(name="const", bufs=1) as cpool,
            tc.tile_pool(name="ps", bufs=6, space=bass.MemorySpace.PSUM) as ppool,
        ):
            xp = cpool.tile([KC, FXB], din)          # ring + ones row
            stage = cpool.tile([NSLOT * NPL, FXB], din)  # input rows 6..11
            wt = cpool.tile([KC, RPC * RX * O], din)     # all 8 rows' weights
            ot = cpool.tile([128, RPC * NG * B], din)    # all 8 rows' outputs

            # --- input loads, all HWDGE on the sync queue, big to small-ish,
            # ordered so pair 0 can start as early as possible ---
            RW = RX * O  # 1920 weight elems per output row per partition
            nc.sync.dma_start(xp[KC - 1:KC, :], x0_d[KC - 1:KC, :])  # ones
            nc.sync.dma_start(wt[:, 0:2 * RW], wh_d[:, 0:2 * RW])
            nc.sync.dma_start(xp[0:5 * NPL, :], x0_d[0:5 * NPL, :])  # rows 0-4
            nc.sync.dma_start(xp[5 * NPL:6 * NPL, :],
                              x0_d[5 * NPL:6 * NPL, :])              # row 5
            nc.sync.dma_start(stage[0:2 * NPL, :], x1_d[0:2 * NPL, :])  # r6,7
            nc.sync.dma_start(wt[:, 2 * RW:4 * RW], wh_d[:, 2 * RW:4 * RW])
            nc.sync.dma_start(stage[2 * NPL:6 * NPL, :],
                              x1_d[2 * NPL:6 * NPL, :])              # r8-11
            nc.sync.dma_start(wt[:, 4 * RW:6 * RW], wh_d[:, 4 * RW:6 * RW])
            nc.sync.dma_start(wt[:, 6 * RW:8 * RW], wh_d[:, 6 * RW:8 * RW])

            for kk in range(RPC):
                m, second = kk // 2, kk % 2
                for ci, (g0, gn) in enumerate(CHUNKS):
                    pt = ppool.tile([128, 4 * B], f32)
                    for gs in range(gn):
                        for j in range(4):
                            xpos = (g0 + gs) * 4 + j
                            nc.tensor.matmul(
                                pt[32 * j:32 * (j + 1), gs * B:(gs + 1) * B],
                                wt[:, (kk * RX + xpos) * O:(kk * RX + xpos + 1) * O],
                                xp[:, xpos * B:(xpos + 1) * B],
                                tile_position=(0, 32 * j),
                            )
                    if second and m < 3:
                        # ring advance: slots 2m,2m+1 <- input rows 2m+6,2m+7,
                        # column range matching this chunk's just-freed reads
                        p0, p1 = 2 * m * NPL, (2 * m + 2) * NPL
                        f0 = g0 * 4 * B
                        f1 = (g0 + gn) * 4 * B
                        nc.sync.dma_start(xp[p0:p1, f0:f1],
                                          stage[p0:p1, f0:f1])
                    dst = ot[:, kk * NG * B + g0 * B:kk * NG * B + (g0 + gn) * B]
                    if ci % 2 == 0:
                        nc.vector.tensor_copy(dst, pt[:, :gn * B])
                    else:
                        nc.scalar.copy(dst, pt[:, :gn * B])
                nc.scalar.dma_start(
                    oc_d[kk], ot[:, kk * NG * B:(kk + 1) * NG * B])

    nc.compile()
    return nc


def _get_nc():
    if "nc" not in _cache:
        _cache["nc"] = _build()
    return _cache["nc"]


def _prep_inputs(x, W, b):
    import ml_dtypes
    bf = ml_dtypes.bfloat16
    x = np.asarray(x, np.float32)
    W = np.asarray(W, np.float32)
    b = np.asarray(b, np.float32)
    xh = np.zeros((PADH, C, WIDTH, B), np.float32)
    xh[:H] = x.transpose(2, 1, 3, 0)  # [row, c, w, batch]
    # patch planes: xpr_full[r, c*KW+dx, x*B+b] = xh[r, c, x+dx, b]
    xpr_full = np.zeros((PADH, C, KW, RX, B), np.float32)
    for dx in range(KW):
        xpr_full[:, :, dx] = xh[:, :, dx:dx + RX]
    xpr_full = xpr_full.reshape(PADH, NPL, FXB).astype(bf)
    Wfull = W.transpose(0, 3, 1, 2)  # [RY, K, RX, O]
    ones_row = np.ones((1, FXB), bf)
    in_maps = []
    for i in range(NCORES):
        whc = np.zeros((RPC, KC, RX, O), np.float32)
        for k in range(RPC):
            y = RPC * i + k
            if y < RY:
                w5 = Wfull[y].reshape(C, KH, KW, RX, O)  # (c, dy, dx, x, o)
                for dy in range(KH):
                    s = (k + dy) % NSLOT  # input row k+dy lives in slot (k+dy)%6
                    whc[k, s * NPL:(s + 1) * NPL] = \
                        w5[:, dy].reshape(NPL, RX, O)
                whc[k, KC - 1] = b[y]
        x0 = np.concatenate(
            [xpr_full[RPC * i:RPC * i + NSLOT].reshape(NSLOT * NPL, FXB),
             ones_row], axis=0)
        in_maps.append({
            "x0": np.ascontiguousarray(x0),
            "x1": np.ascontiguousarray(
                xpr_full[RPC * i + NSLOT:RPC * i + 2 * NSLOT]
                .reshape(NSLOT * NPL, FXB)),
            "wh": np.ascontiguousarray(
                whc.transpose(1, 0, 2, 3).reshape(KC, RPC * RX * O)).astype(bf),
        })
    return in_maps


def kernel(x, W, b):
    from concourse.bass_utils import run_bass_kernel_spmd

    nc = _get_nc()
    in_maps = _prep_inputs(x, W, b)
    br = run_bass_kernel_spmd(nc, in_maps, list(range(NCORES)),
                              **_cache.get("run_kwargs", {}))
    _cache["last_run"] = br
    oc = np.stack([np.asarray(br.results[i]["oc"]) for i in range(NCORES)])
    # oc: [i, k, p=32j+o, g*B+b] -> out[b, o, y=8i+k, x=g*4+j]
    oc = oc.reshape(NCORES * RPC, 4, O, NG, B).astype(np.float32)
    out = oc.transpose(4, 2, 0, 3, 1).reshape(B, O, NCORES * RPC, RX)
    return np.ascontiguousarray(out[:, :, :RY, :])


# revision 10
# speedup vs baseline: 1.1673x; 1.1070x over previous
"""Locally-connected 2D layer on 8 Trainium2 NeuronCores.

Problem: x[128,3,64,64] f32, per-position weights W[60,60,32,75], bias b[60,60,32]
  out[b,o,y,x] = sum_k patches[b,y,x,k] * W[y,x,o,k] + b[y,x,o],  k=(c,dy,dx)

Strategy (spatial sharding over output rows, 8 rows/core, memory-regime):
  - mod-8 ring of input-row patch planes on SBUF partitions 0..119 (8 slots x
    15 (c,dx)-planes), ones row at partition 120 -> contraction K=121.  Input
    row r lives in slot r%8; output row k multiplies slots k..k+4 (mod 8) with
    nonzero weights and the other 45 plane rows with zeros, so the rhs window
    is always the full fixed [0,121) partition range - no ring rotation, no
    wraparound, no SBUF->SBUF copies.
  - W is stored per-row UNPADDED in HBM ([75,1920] per output row) and DMA'd
    into a zero-memset [121, 8*1920] tile at the row's (possibly wrapped)
    partition stripes; bias is one [1, 8*1920] row at partition 120 (ones).
  - Ring advances (input rows 8..11 -> slots 0..3) are plain HBM loads with
    ~3 output rows of scheduling slack, column-halved and issued right after
    the last reader's matmuls.
  - All DMAs are large-ish and spread over the sync/scalar/gpsimd queues for
    parallel dispatch + deeper SDMA pipelining (~250 GB/s aggregate).
  - Per output row: 15 groups of 4 column-tiled matmuls (lhsT=W[121,32],
    rhs=XP[121,128] -> out[32o,128b] at PSUM partitions 32j); PSUM->SBUF
    copies (f32->bf16) rotate over vector/scalar/gpsimd; bf16 stores per pair.
"""

import numpy as np

B, C, H, WIDTH = 128, 3, 64, 64
KH = KW = 5
RY = RX = 60
O = 32
NCORES = 8
RPC = 8             # output rows computed per core (8*8=64, last 4 dropped)
NSLOT = 8           # ring slots; contraction = 8*15 + 1(ones) = 121
NPL = KW * C        # 15 planes per input row
KC = NSLOT * NPL + 1  # 121 contraction rows
PADH = NCORES * RPC + KH - 1  # 68
NG = 15             # groups of 4 x-positions per row
CHUNKS = ((0, 4), (4, 4), (8, 4), (12, 3))  # (first group, n groups) per PSUM chunk
FXB = RX * B        # 7680 elements per patch plane
RW = RX * O         # 1920 weight elems per output row per partition

_cache = {}


def _build():
    import concourse.bass as bass
    import concourse.bacc as bacc
    import concourse.tile as tile
    import concourse.mybir as mybir

    f32 = mybir.dt.float32
    din = mybir.dt.bfloat16
    nc = bacc.Bacc("TRN2", target_bir_lowering=False, debug=False,
                   num_devices=NCORES)
    x0_d = nc.dram_tensor("x0", [KC, FXB], din, kind="ExternalInput")
    xn_d = nc.dram_tensor("xn", [4, NPL, FXB], din, kind="ExternalInput")
    wh_d = nc.dram_tensor("wh", [RPC, 75, RW], din, kind="ExternalInput")
    wb_d = nc.dram_tensor("wb", [1, RPC * RW], din, kind="ExternalInput")
    oc_d = nc.dram_tensor("oc", [4, 128, 2 * NG * B], din, kind="ExternalOutput")

    with tile.TileContext(nc) as tc:
        with (
            tc.tile_pool(name="const", bufs=1) as cpool,
            tc.tile_pool(name="ps", bufs=6, space=bass.MemorySpace.PSUM) as ppool,
        ):
            xp = cpool.tile([KC, FXB], din)       # 8 slots + ones row
            wt = cpool.tile([KC, RPC * RW], din)  # all 8 rows' weights (padded)
            ots = [cpool.tile([128, 2 * NG * B], din, name=f"ot{m}")
                   for m in range(4)]

            # zero the weight tile first (covers the 45 pad rows per column
            # block); W stripes below overwrite their live partition ranges
            nc.vector.memset(wt[:], 0.0)

            # --- input loads: sync = x-planes + upper W rows, scalar = lower
            # W rows (needed first), big-ish descriptors, 3 queues total ---
            nc.scalar.dma_start(wt[0:75, 0:RW], wh_d[0])  # W row 0 (stripe 0)
            nc.sync.dma_start(xp[0:75, :], x0_d[0:75, :])       # rows 0-4
            nc.sync.dma_start(xp[75:KC, :], x0_d[75:KC, :])     # rows 5-7+ones
            nc.sync.dma_start(wt[KC - 1:KC, :], wb_d[:])        # bias row
            for k in range(1, RPC):
                eng = nc.scalar if k < 4 else nc.sync
                p0 = NPL * k
                l1 = min(p0 + 75, NSLOT * NPL) - p0  # live rows before wrap
                eng.dma_start(wt[p0:p0 + l1, k * RW:(k + 1) * RW],
                              wh_d[k, 0:l1])
                if l1 < 75:
                    eng.dma_start(wt[0:75 - l1, k * RW:(k + 1) * RW],
                                  wh_d[k, l1:75])

            for kk in range(RPC):
                m, second = kk // 2, kk % 2
                ot = ots[m]
                for ci, (g0, gn) in enumerate(CHUNKS):
                    pt = ppool.tile([128, 4 * B], f32)
                    for gs in range(gn):
                        for j in range(4):
                            xpos = (g0 + gs) * 4 + j
                            nc.tensor.matmul(
                                pt[32 * j:32 * (j + 1), gs * B:(gs + 1) * B],
                                wt[:, (kk * RX + xpos) * O:(kk * RX + xpos + 1) * O],
                                xp[:, xpos * B:(xpos + 1) * B],
                                tile_position=(0, 32 * j),
                            )
                    if 3 <= kk < 7 and ci in (1, 3):
                        # ring advance: slot kk-3 <- input row kk+5, issued
                        # after the slot's last reader (this row's) matmuls
                        # for the matching column half
                        s = kk - 3
                        f0 = 0 if ci == 1 else 8 * B
                        f1 = 8 * B if ci == 1 else NG * B
                        nc.gpsimd.dma_start(
                            xp[s * NPL:(s + 1) * NPL, f0 * 4:f1 * 4],
                            xn_d[s][:, f0 * 4:f1 * 4])
                    dst = ot[:, second * NG * B + g0 * B:
                             second * NG * B + (g0 + gn) * B]
                    if ci % 2:
                        nc.scalar.copy(dst, pt[:, :gn * B])
                    else:
                        nc.vector.tensor_copy(dst, pt[:, :gn * B])
                if second:
                    nc.scalar.dma_start(oc_d[m], ot[:])

    nc.compile()
    return nc


def _get_nc():
    if "nc" not in _cache:
        _cache["nc"] = _build()
    return _cache["nc"]


def _prep_inputs(x, W, b):
    import ml_dtypes
    bf = ml_dtypes.bfloat16
    x = np.asarray(x, np.float32)
    W = np.asarray(W, np.float32)
    b = np.asarray(b, np.float32)
    xh = np.zeros((PADH, C, WIDTH, B), np.float32)
    xh[:H] = x.transpose(2, 1, 3, 0)  # [row, c, w, batch]
    # patch planes: xpr_full[r, c*KW+dx, x*B+b] = xh[r, c, x+dx, b]
    xpr_full = np.zeros((PADH, C, KW, RX, B), np.float32)
    for dx in range(KW):
        xpr_full[:, :, dx] = xh[:, :, dx:dx + RX]
    xpr_full = xpr_full.reshape(PADH, NPL, FXB).astype(bf)
    Wfull = W.transpose(0, 3, 1, 2)  # [RY, 75, RX, O]
    ones_row = np.ones((1, FXB), bf)
    in_maps = []
    for i in range(NCORES):
        # per-row unpadded W: [RPC, 75, RX*O], partition order (dy, c, dx)
        whc = np.zeros((RPC, 75, RX, O), np.float32)
        wbc = np.zeros((RPC, RX, O), np.float32)
        for k in range(RPC):
            y = RPC * i + k
            if y < RY:
                w5 = Wfull[y].reshape(C, KH, KW, RX, O)  # (c, dy, dx, x, o)
                whc[k] = w5.transpose(1, 0, 2, 3, 4).reshape(75, RX, O)
                wbc[k] = b[y]
        in_maps.append({
            "x0": np.concatenate(
                [xpr_full[RPC * i:RPC * i + NSLOT].reshape(NSLOT * NPL, FXB),
                 ones_row], axis=0),
            "xn": np.ascontiguousarray(
                xpr_full[RPC * i + NSLOT:RPC * i + NSLOT + 4]),
            "wh": np.ascontiguousarray(whc.reshape(RPC, 75, RW)).astype(bf),
            "wb": np.ascontiguousarray(wbc.reshape(1, RPC * RW)).astype(bf),
        })
    return in_maps


def kernel(x, W, b):
    from concourse.bass_utils import run_bass_kernel_spmd

    nc = _get_nc()
    in_maps = _prep_inputs(x, W, b)
    br = run_bass_kernel_spmd(nc, in_maps, list(range(NCORES)),
                              **_cache.get("run_kwargs", {}))
    _cache["last_run"] = br
    oc = np.stack([np.asarray(br.results[i]["oc"]) for i in range(NCORES)])
    # oc: [i, m, p=32j+o, k2*NG*B + g*B + b] -> out[b, o, y=8i+2m+k2, x=4g+j]
    oc = oc.reshape(NCORES, 4, 4, O, 2, NG, B).astype(np.float32)
    out = oc.transpose(6, 3, 0, 1, 4, 5, 2).reshape(B, O, NCORES * RPC, RX)
    return np.ascontiguousarray(out[:, :, :RY, :])


# revision 17
# speedup vs baseline: 1.1979x; 1.0262x over previous
"""Locally-connected 2D layer on 8 Trainium2 NeuronCores.

Problem: x[128,3,64,64] f32, per-position weights W[60,60,32,75], bias b[60,60,32]
  out[b,o,y,x] = sum_k patches[b,y,x,k] * W[y,x,o,k] + b[y,x,o],  k=(c,dy,dx)

Strategy (spatial sharding over output rows, 8 rows/core, memory-regime):
  - mod-8 ring of input-row patch planes on SBUF partitions 0..119 (8 slots x
    15 (c,dx)-planes), ones row at partition 120 -> contraction K=121.  Input
    row r lives in slot r%8; output row k multiplies slots k..k+4 (mod 8) with
    nonzero weights and the other 45 plane rows with zeros, so the rhs window
    is always the full fixed [0,121) partition range - no ring rotation, no
    wraparound, no SBUF->SBUF copies.
  - W is stored per-row UNPADDED in HBM ([75,1920] per output row) and DMA'd
    into a zero-memset [121, 8*1920] tile at the row's (possibly wrapped)
    partition stripes; bias is one [1, 8*1920] row at partition 120 (ones).
  - Ring advances (input rows 8..11 -> slots 0..3) are plain HBM loads with
    ~3 output rows of scheduling slack, column-halved and issued right after
    the last reader's matmuls.
  - All DMAs are large-ish and spread over the sync/scalar/gpsimd queues for
    parallel dispatch + deeper SDMA pipelining (~250 GB/s aggregate).
  - Per output row: 15 groups of 4 column-tiled matmuls (lhsT=W[121,32],
    rhs=XP[121,128] -> out[32o,128b] at PSUM partitions 32j); PSUM->SBUF
    copies (f32->bf16) rotate over vector/scalar/gpsimd; bf16 stores per pair.
"""

import numpy as np

B, C, H, WIDTH = 128, 3, 64, 64
KH = KW = 5
RY = RX = 60
O = 32
NCORES = 8
RPC = 8             # output rows computed per core (8*8=64, last 4 dropped)
NSLOT = 8           # ring slots; contraction = 8*15 + 1(ones) = 121
NPL = KW * C        # 15 planes per input row
KC = NSLOT * NPL + 1  # 121 contraction rows
PADH = NCORES * RPC + KH - 1  # 68
NG = 15             # groups of 4 x-positions per row
CHUNKS = ((0, 4), (4, 4), (8, 4), (12, 3))  # (first group, n groups) per PSUM chunk
FXB = RX * B        # 7680 elements per patch plane
RW = RX * O         # 1920 weight elems per output row per partition

_cache = {}


def _build():
    import concourse.bass as bass
    import concourse.bacc as bacc
    import concourse.tile as tile
    import concourse.mybir as mybir

    f32 = mybir.dt.float32
    din = mybir.dt.bfloat16
    nc = bacc.Bacc("TRN2", target_bir_lowering=False, debug=False,
                   num_devices=NCORES)
    x0_d = nc.dram_tensor("x0", [KC, FXB], din, kind="ExternalInput")
    xn_d = nc.dram_tensor("xn", [2, 2 * NPL, FXB], din, kind="ExternalInput")
    wh_d = nc.dram_tensor("wh", [RPC, 75, RW], din, kind="ExternalInput")
    wb_d = nc.dram_tensor("wb", [1, RPC * RW], din, kind="ExternalInput")
    oc_d = nc.dram_tensor("oc", [4, 128, 2 * NG * B], din, kind="ExternalOutput")

    with tile.TileContext(nc) as tc:
        with (
            tc.tile_pool(name="const", bufs=1) as cpool,
            tc.tile_pool(name="ps", bufs=6, space=bass.MemorySpace.PSUM) as ppool,
        ):
            xp = cpool.tile([KC, FXB], din)       # 8 slots + ones row
            wt = cpool.tile([KC, RPC * RW], din)  # all 8 rows' weights (padded)
            ots = [cpool.tile([128, 2 * NG * B], din, name=f"ot{m}")
                   for m in range(4)]

            # zero the pad rows of each W column block, alternating engines so
            # blocks complete pipelined (~1.6us each) instead of one 13us
            # DVE memset gating every W load
            for k in range(RPC):
                eng = nc.vector if k % 2 == 0 else nc.gpsimd
                eng.memset(wt[:, k * RW:(k + 1) * RW], 0.0)

            # --- input loads: sync = x-planes + odd W rows (+ advances later),
            # scalar = even W rows (+ stores later) ---
            nc.scalar.dma_start(wt[0:75, 0:RW], wh_d[0])  # W row 0 (stripe 0)
            nc.sync.dma_start(xp[0:75, :], x0_d[0:75, :])       # rows 0-4
            nc.sync.dma_start(xp[75:KC, :], x0_d[75:KC, :])     # rows 5-7+ones
            nc.sync.dma_start(wt[KC - 1:KC, :], wb_d[:])        # bias row
            for k in range(1, RPC):
                eng = nc.sync if k % 2 else nc.scalar
                p0 = NPL * k
                l1 = min(p0 + 75, NSLOT * NPL) - p0  # live rows before wrap
                eng.dma_start(wt[p0:p0 + l1, k * RW:(k + 1) * RW],
                              wh_d[k, 0:l1])
                if l1 < 75:
                    eng.dma_start(wt[0:75 - l1, k * RW:(k + 1) * RW],
                                  wh_d[k, l1:75])

            for kk in range(RPC):
                m, second = kk // 2, kk % 2
                ot = ots[m]
                for ci, (g0, gn) in enumerate(CHUNKS):
                    pt = ppool.tile([128, 4 * B], f32)
                    for gs in range(gn):
                        for j in range(4):
                            xpos = (g0 + gs) * 4 + j
                            nc.tensor.matmul(
                                pt[32 * j:32 * (j + 1), gs * B:(gs + 1) * B],
                                wt[:, (kk * RX + xpos) * O:(kk * RX + xpos + 1) * O],
                                xp[:, xpos * B:(xpos + 1) * B],
                                tile_position=(0, 32 * j),
                            )
                    if kk in (2, 3) and ci in (1, 3):
                        # ring advance: slots 0,1 (rows 8,9) after row 2's
                        # reads, slots 2,3 (rows 10,11) after row 3's, in
                        # column halves matching the chunks just retired
                        s = kk - 2
                        f0 = 0 if ci == 1 else 8 * 4 * B
                        f1 = 8 * 4 * B if ci == 1 else NG * 4 * B
                        nc.sync.dma_start(
                            xp[s * 2 * NPL:(s + 1) * 2 * NPL, f0:f1],
                            xn_d[s][:, f0:f1])
                    dst = ot[:, second * NG * B + g0 * B:
                             second * NG * B + (g0 + gn) * B]
                    if ci % 2:
                        nc.scalar.copy(dst, pt[:, :gn * B])
                    else:
                        nc.vector.tensor_copy(dst, pt[:, :gn * B])
                if kk >= 6:
                    # split the final pair's store per row to shorten the tail
                    nc.scalar.dma_start(
                        oc_d[m][:, second * NG * B:(second + 1) * NG * B],
                        ot[:, second * NG * B:(second + 1) * NG * B])
                elif second:
                    nc.scalar.dma_start(oc_d[m], ot[:])

    nc.compile()
    return nc


def _get_nc():
    if "nc" not in _cache:
        _cache["nc"] = _build()
    return _cache["nc"]


def _prep_inputs(x, W, b):
    import ml_dtypes
    bf = ml_dtypes.bfloat16
    x = np.asarray(x, np.float32)
    W = np.asarray(W, np.float32)
    b = np.asarray(b, np.float32)
    xh = np.zeros((PADH, C, WIDTH, B), np.float32)
    xh[:H] = x.transpose(2, 1, 3, 0)  # [row, c, w, batch]
    # patch planes: xpr_full[r, c*KW+dx, x*B+b] = xh[r, c, x+dx, b]
    xpr_full = np.zeros((PADH, C, KW, RX, B), np.float32)
    for dx in range(KW):
        xpr_full[:, :, dx] = xh[:, :, dx:dx + RX]
    xpr_full = xpr_full.reshape(PADH, NPL, FXB).astype(bf)
    Wfull = W.transpose(0, 3, 1, 2)  # [RY, 75, RX, O]
    ones_row = np.ones((1, FXB), bf)
    in_maps = []
    for i in range(NCORES):
        # per-row unpadded W: [RPC, 75, RX*O], partition order (dy, c, dx)
        whc = np.zeros((RPC, 75, RX, O), np.float32)
        wbc = np.zeros((RPC, RX, O), np.float32)
        for k in range(RPC):
            y = RPC * i + k
            if y < RY:
                w5 = Wfull[y].reshape(C, KH, KW, RX, O)  # (c, dy, dx, x, o)
                whc[k] = w5.transpose(1, 0, 2, 3, 4).reshape(75, RX, O)
                wbc[k] = b[y]
        in_maps.append({
            "x0": np.concatenate(
                [xpr_full[RPC * i:RPC * i + NSLOT].reshape(NSLOT * NPL, FXB),
                 ones_row], axis=0),
            "xn": np.ascontiguousarray(
                xpr_full[RPC * i + NSLOT:RPC * i + NSLOT + 4]
                .reshape(2, 2 * NPL, FXB)),
            "wh": np.ascontiguousarray(whc.reshape(RPC, 75, RW)).astype(bf),
            "wb": np.ascontiguousarray(wbc.reshape(1, RPC * RW)).astype(bf),
        })
    return in_maps


def kernel(x, W, b):
    from concourse.bass_utils import run_bass_kernel_spmd

    nc = _get_nc()
    in_maps = _prep_inputs(x, W, b)
    br = run_bass_kernel_spmd(nc, in_maps, list(range(NCORES)),
                              **_cache.get("run_kwargs", {}))
    _cache["last_run"] = br
    oc = np.stack([np.asarray(br.results[i]["oc"]) for i in range(NCORES)])
    # oc: [i, m, p=32j+o, k2*NG*B + g*B + b] -> out[b, o, y=8i+2m+k2, x=4g+j]
    oc = oc.reshape(NCORES, 4, 4, O, 2, NG, B).astype(np.float32)
    out = oc.transpose(6, 3, 0, 1, 4, 5, 2).reshape(B, O, NCORES * RPC, RX)
    return np.ascontiguousarray(out[:, :, :RY, :])
